# revision 22
# baseline (speedup 1.0000x reference)
"""Bass/Trainium2 kernel for nn_MultiHeadAttention (B=4, S=2048, E=512, H=8, dk=dv=8).

Sharding: 8 cores = (batch b, head-half hh).  Core 2b+hh computes causal
attention for batch b over heads [4hh, 4hh+4) for all 2048 queries, applies
its half of the output projection, and returns a partial output transposed
[E, S] in bf16.  Host sums the two partials per batch, transposes, and adds
bo.

v2 design notes (vs the f32r baseline):
  - All inputs are shipped bf16; matmuls for scores / A@V / projections /
    out-proj run in bf16 (1 cycle/row on the PE vs 2 for fp32 HIGH mode).
  - Causal masking is multiplicative AFTER exp (ats *= 0/1 mask, bf16
    SBUF 4x-mode DVE) instead of additive -1e30 on PSUM scores.
  - Softmax reciprocal via reciprocal_approx_fast on 4 strided partitions
    (the old per-head full reciprocal was 53us of DVE).
  - Per-q-chunk loop fuses projection of chunk c+1 into the attention
    block stream of chunk c so ACT (the bottleneck engine, exp) never
    starves while the PE does projections.
  - Emission order per block i: scores(i) -> exp(i) -> av(i-1), keeping
    score matmuls ahead of the ACT stream.
"""

import math

import numpy as np

B, S, E, H = 4, 2048, 512, 8
DK_H = DV_H = 8
NCORES = 8
HPC = H // 2  # heads per core = 4
SCALE = 1.0 / math.sqrt(DK_H)
NQC = S // 512  # q chunks of 512
NTB = S // 128  # t blocks of 128
ECH = E // 128  # e chunks of 128

_cache: dict = {}


def _apply_tile_patch():
    """walrus in this image allows only one sync-wait per Drain; split the
    TileContext tail drain's waits across a chain of drains."""
    import concourse.mybir as mybir
    from concourse import tile
    from concourse.vector_clock import ScopedClock

    if getattr(tile.TileContext._drain_and_barrier, "_split_patch", False):
        return

    def _drain_and_barrier_split(self, tick_clock, wait_clock):
        drain_inst = self.nc.sync.drain()
        wait_clock.add_sem_waits(
            drain_inst.ins, ScopedClock({None: tick_clock.global_clock})
        )
        si = drain_inst.ins.sync_info
        if si is not None and si.on_wait and len(si.on_wait) > 1:
            waits = list(si.on_wait)
            si.on_wait = waits[:1]
            for entry in waits[1:]:
                extra = self.nc.sync.drain()
                extra.ins.sync_info = mybir.SyncInfo(on_wait=[entry], on_update=[])
        self.nc.all_engine_barrier()
        assert self.sems is not None
        popped = self.nc._tile_sem_poison_stack.pop()
        assert popped is self._sem_poison
        self.nc.clear_and_free_semaphores(list(self.sems.allocated().values()))
        self.nc.all_engine_barrier()

    _drain_and_barrier_split._split_patch = True
    tile.TileContext._drain_and_barrier = _drain_and_barrier_split


def _split_multi_waits(nc):
    """walrus in this image allows only one sync-wait per instruction;
    move excess waits onto single-wait NOPs inserted just before."""
    import concourse.mybir as mybir

    for blk in nc.m.functions[0].blocks:
        out = []
        for inst in blk.instructions:
            si = getattr(inst, "sync_info", None)
            if si is not None and si.on_wait and len(si.on_wait) > 1:
                waits = list(si.on_wait)
                for i, entry in enumerate(waits[:-1]):
                    out.append(
                        mybir.InstNoOp(
                            name=f"{inst.name}_w{i}",
                            engine=inst.engine,
                            ins=[],
                            outs=[],
                            bass_nofuse=True,
                            sync_info=mybir.SyncInfo(
                                on_wait=[entry], on_update=[]
                            ),
                        )
                    )
                si.on_wait = waits[-1:]
            out.append(inst)
        blk.instructions = out


def _build(for_sim: bool = False):
    import concourse.bass as bassmod
    import concourse.mybir as mybir
    from concourse import tile

    if not for_sim:
        _apply_tile_patch()
    f32 = mybir.dt.float32
    bf16 = mybir.dt.bfloat16
    f32r = mybir.dt.float32r
    Exp = mybir.ActivationFunctionType.Exp

    def rr(ap):
        return ap.bitcast(f32r)

    nc = bassmod.Bass()
    qT = nc.declare_dram_parameter("qT", [E, S], bf16, isOutput=False)
    kT = nc.declare_dram_parameter("kT", [E, S], bf16, isOutput=False)
    vT = nc.declare_dram_parameter("vT", [E, S], bf16, isOutput=False)
    wq = nc.declare_dram_parameter("wq", [E, 128], bf16, isOutput=False)
    wk = nc.declare_dram_parameter("wk", [E, 128], bf16, isOutput=False)
    wv = nc.declare_dram_parameter("wv", [E, HPC * 9], bf16, isOutput=False)
    wo = nc.declare_dram_parameter("wo", [128, E], bf16, isOutput=False)
    msk = nc.declare_dram_parameter("msk", [128, 2 * 128], bf16, isOutput=False)
    out = nc.declare_dram_parameter("out", [E, S], bf16, isOutput=True)

    with tile.TileContext(nc) as tc:
        with (
            tc.tile_pool(name="singles", bufs=1) as singles,
            tc.tile_pool(name="loads", bufs=6) as loads,
            tc.tile_pool(name="abuf", bufs=4) as abuf,
            tc.tile_pool(name="outs", bufs=3) as outs,
            tc.tile_pool(name="norms", bufs=2) as norms,
            tc.tile_pool(name="ps_sc", bufs=2, space="PSUM") as ps_sc,
            tc.tile_pool(name="ps_av", bufs=2, space="PSUM") as ps_av,
            tc.tile_pool(name="ps_misc", bufs=2, space="PSUM") as ps_misc,
        ):
            # ---- resident tensors -------------------------------------
            wq_sb = singles.tile([128, ECH, 128], bf16, tag="wq")
            wk_sb = singles.tile([128, ECH, 128], bf16, tag="wk")
            wv_sb = singles.tile([128, ECH, HPC * 9], bf16, tag="wv")
            wo_sb = singles.tile([128, ECH, 128], bf16, tag="wo")
            msk_sb = singles.tile([128, 2, 128], bf16, tag="msk")
            nc.sync.dma_start(out=wq_sb, in_=wq.rearrange("(c p) m -> p c m", p=128))
            nc.sync.dma_start(out=wk_sb, in_=wk.rearrange("(c p) m -> p c m", p=128))
            nc.sync.dma_start(out=wv_sb, in_=wv.rearrange("(c p) m -> p c m", p=128))
            nc.sync.dma_start(out=wo_sb, in_=wo.rearrange("p (c m) -> p c m", c=ECH))
            nc.sync.dma_start(out=msk_sb, in_=msk.rearrange("p (g n) -> p g n", g=2))

            KTs = singles.tile([128, S], bf16, tag="KTs")
            QTs = singles.tile([128, S], bf16, tag="QTs")
            Vsb = singles.tile([128, NTB, HPC, 9], bf16, tag="Vsb")
            onorm = singles.tile([128, S], bf16, tag="onorm")

            nc.vector.memset(onorm, 0.0)
            nc.vector.memset(Vsb[:, :, :, 0:1], 1.0)

            # PE warmup: ~5us of dummy matmuls so the HAM clock-gate opens
            # before real work arrives (DMA waits would otherwise keep the
            # PE cold).  Also hoist the ACT exp-table load.
            warm_sb = singles.tile([128, 512], bf16, tag="warm")
            nc.vector.memset(warm_sb, 0.0)
            warm_ps = ps_misc.tile([128, 512], f32, tag="ps", name="warm")
            for i in range(14):
                nc.tensor.matmul(
                    warm_ps, warm_sb[:, 0:128], warm_sb,
                    start=(i == 0), stop=(i == 13),
                )
            nc.scalar.activation(
                warm_sb[0:1, 0:2].bitcast(f32), warm_ps[0:1, 0:1], Exp,
            )

            # ---- input DMAs (prefetched, flow-controlled by pool) -----
            ld_tiles = {}

            def emit_loads(c):
                cs = slice(c * 512, (c + 1) * 512)
                for name, dram in (("q", qT), ("k", kT), ("v", vT)):
                    t = loads.tile([128, ECH, 512], bf16, tag="ld")
                    nc.sync.dma_start(
                        out=t,
                        in_=dram[:, cs].rearrange("(c p) m -> p c m", p=128),
                    )
                    ld_tiles[(name, c)] = t

            # ---- projection steps for one chunk -----------------------
            def proj_steps(c):
                cs = slice(c * 512, (c + 1) * 512)

                def q_mm():
                    qt_ps = ps_misc.tile([128, 512], f32, tag="ps", name=f"qp{c}")
                    src = ld_tiles[("q", c)]
                    for e in range(ECH):
                        nc.tensor.matmul(
                            qt_ps, wq_sb[:, e, :], src[:, e, :],
                            start=(e == 0), stop=(e == ECH - 1),
                        )
                    ld_tiles[(f"qps", c)] = qt_ps

                def q_cp():
                    nc.vector.tensor_copy(QTs[:, cs], ld_tiles[(f"qps", c)])

                def k_mm():
                    kt_ps = ps_misc.tile([128, 512], f32, tag="ps", name=f"kp{c}")
                    src = ld_tiles[("k", c)]
                    for e in range(ECH):
                        nc.tensor.matmul(
                            kt_ps, wk_sb[:, e, :], src[:, e, :],
                            start=(e == 0), stop=(e == ECH - 1),
                        )
                    ld_tiles[(f"kps", c)] = kt_ps

                def k_cp():
                    nc.vector.tensor_copy(KTs[:, cs], ld_tiles[(f"kps", c)])

                def v_tb(tb_local):
                    def go():
                        tb = 4 * c + tb_local
                        src = ld_tiles[("v", c)]
                        v_ps = ps_misc.tile(
                            [128, HPC * 9], f32, tag="ps", name=f"vp{c}_{tb_local}"
                        )
                        for e in range(ECH):
                            nc.tensor.matmul(
                                v_ps,
                                src[:, e, tb_local * 128:(tb_local + 1) * 128],
                                wv_sb[:, e, :],
                                start=(e == 0), stop=(e == ECH - 1),
                            )
                        dst = Vsb[:, tb, :, 1:9]
                        vsrc = v_ps.rearrange("p (h n) -> p h n", n=9)[:, :, 1:9]
                        nc.vector.tensor_copy(dst, vsrc)

                    return go

                return [q_mm, q_cp, k_mm, k_cp, v_tb(0), v_tb(1), v_tb(2), v_tb(3)]

            # ---- deferred per-chunk tail (normalize + out-projection) --
            def tail_steps(c, av):
                cs = slice(c * 512, (c + 1) * 512)
                den4 = norms.tile([128, 512], f32, tag="den", name=f"den{c}")
                recin = norms.tile([128, 16], f32, tag="ri", name=f"ri{c}")
                recout = norms.tile([128, 16], f32, tag="ro", name=f"ro{c}")
                recrow = norms.tile([128, 512], f32, tag="rr", name=f"rr{c}")
                rep_sb = norms.tile([128, 512], f32, tag="rep", name=f"rep{c}")

                def den_cp():
                    # drain denominator rows out of PSUM (lane-preserving)
                    for h in range(HPC):
                        nc.vector.tensor_copy(
                            den4[32 * h:32 * h + 1, :],
                            av[32 * h:32 * h + 1, :],
                        )

                def recip_chain():
                    # spread each 512-wide row over 32 partitions so one
                    # cheap reciprocal covers all heads at 16 cols/lane,
                    # then restore rows and replicate to 9 rows per head
                    for h in range(HPC):
                        nc.sync.dma_start(
                            out=recin[32 * h:32 * h + 32, :],
                            in_=den4[32 * h:32 * h + 1, :].rearrange(
                                "o (p m) -> o p m", p=32
                            ),
                        )
                    nc.vector.reciprocal(recout, recin)
                    for h in range(HPC):
                        nc.sync.dma_start(
                            out=recrow[32 * h:32 * h + 1, :].rearrange(
                                "o (p m) -> o p m", p=32
                            ),
                            in_=recout[32 * h:32 * h + 32, :],
                        )
                    for h in range(HPC):
                        nc.sync.dma_start(
                            out=rep_sb[32 * h:32 * h + 9, :],
                            in_=recrow[32 * h:32 * h + 1, :]
                            .rearrange("p (x m) -> p x m", x=1)
                            .broadcast_to((1, 9, 512)),
                        )

                def muls():
                    for h in range(HPC):
                        nc.vector.tensor_mul(
                            onorm[32 * h:32 * h + 9, cs],
                            av[32 * h:32 * h + 9, :],
                            rep_sb[32 * h:32 * h + 9, :],
                        )

                def oproj(e):
                    def go():
                        f_ps = ps_misc.tile([128, 512], f32, tag="ps",
                                            name=f"f{c}_{e}")
                        nc.tensor.matmul(
                            f_ps, wo_sb[:, e, :], onorm[:, cs],
                            start=True, stop=True,
                        )
                        fsb = outs.tile([128, 512], bf16, tag="f")
                        nc.vector.tensor_copy(fsb, f_ps)
                        nc.sync.dma_start(
                            out=out[e * 128:(e + 1) * 128, cs], in_=fsb
                        )
                    return go

                return [den_cp, recip_chain, muls,
                        oproj(0), oproj(1), oproj(2), oproj(3)]

            # ---- main fused loop --------------------------------------
            emit_loads(0)
            emit_loads(1)
            for step in proj_steps(0):
                step()

            pending = []  # deferred work consumed inside the block loop
            for c in range(NQC):
                cs = slice(c * 512, (c + 1) * 512)
                if c + 1 < NQC:
                    pending = pending + proj_steps(c + 1)
                if c + 2 < NQC:
                    emit_loads(c + 2)

                av = ps_av.tile([128, 512], f32, tag="av", name=f"av{c}")
                ntb = 4 * (c + 1)
                nsteps = len(pending)
                prev = None  # deferred A@V emission for the previous block
                for tb in range(ntb):
                    d = 128 * tb - 512 * c  # diagonal offset within the chunk
                    vstart = max(d, 0)
                    scs = [
                        ps_sc.tile([128, 2, 512], f32, tag="sc",
                                   name=f"sc{c}_{tb}_0"),
                        ps_sc.tile([128, 2, 512], f32, tag="sc",
                                   name=f"sc{c}_{tb}_1"),
                    ]
                    ats = [
                        abuf.tile([128, 2, 512], bf16, tag="a",
                                  name=f"a{c}_{tb}_0"),
                        abuf.tile([128, 2, 512], bf16, tag="a",
                                  name=f"a{c}_{tb}_1"),
                    ]
                    # scores for block tb (4 heads row-tiled, concurrent)
                    for h in range(HPC):
                        g, j = divmod(h, 2)
                        nc.tensor.matmul(
                            scs[g][:, j, :],
                            KTs[32 * h:32 * h + 8, tb * 128:(tb + 1) * 128],
                            QTs[32 * h:32 * h + 8, cs],
                            start=True, stop=True,
                            tile_position=(32 * h, 0),
                        )
                    # exp (ACT) + causal 0/1 mask on the diagonal block (DVE)
                    for g in range(2):
                        nc.scalar.activation(
                            ats[g][:, :, vstart:512], scs[g][:, :, vstart:512],
                            Exp, scale=SCALE,
                        )
                        if d >= 0:
                            nc.gpsimd.tensor_mul(
                                ats[g][:, :, d:d + 128],
                                ats[g][:, :, d:d + 128],
                                msk_sb,
                            )
                    # previous block's A@V (keeps score MMs ahead of ACT)
                    if prev is not None:
                        prev()
                    # interleaved projection work for chunk c+1
                    lo = (tb * nsteps) // ntb
                    hi = ((tb + 1) * nsteps) // ntb
                    for si in range(lo, hi):
                        pending[si]()

                    def make_av(tb, vstart, ats):
                        def go():
                            for h in range(HPC):
                                g, j = divmod(h, 2)
                                nc.tensor.matmul(
                                    av[32 * h:32 * h + 9, vstart:512],
                                    Vsb[:, tb, h, :],
                                    ats[g][:, j, vstart:512],
                                    start=(tb == 0), stop=(tb == ntb - 1),
                                    tile_position=(0, 32 * h),
                                    skip_group_check=True,
                                )
                        return go

                    prev = make_av(tb, vstart, ats)
                prev()

                # tail work (normalize + out-proj) is deferred into the
                # next chunk's block loop so it never head-of-line blocks
                # the exp stream; the last chunk's tail runs here.
                pending = tail_steps(c, av)
                if c == NQC - 1:
                    for step in pending:
                        step()
                    pending = []
    if not for_sim:
        _split_multi_waits(nc)
    return nc


def _prep_inputs(query, key, value, Wq, Wk, Wv, Wo):
    """Build the 8 per-core input maps (host-side sharding/layout)."""
    import ml_dtypes

    bf = ml_dtypes.bfloat16
    qTs = [np.ascontiguousarray(query[b].T).astype(bf) for b in range(B)]
    kTs = [np.ascontiguousarray(key[b].T).astype(bf) for b in range(B)]
    vTs = [np.ascontiguousarray(value[b].T).astype(bf) for b in range(B)]

    # 0/1 upper-triangular (t_local <= q_local) mask, duplicated for the
    # two 2-head groups
    mask01 = (np.arange(128)[:, None] <= np.arange(128)[None, :]).astype(bf)
    msk2 = np.ascontiguousarray(np.tile(mask01, (1, 2)))

    in_maps = []
    for core in range(NCORES):
        b, hh = divmod(core, 2)
        wq_p = np.zeros((E, 128), np.float32)
        wk_p = np.zeros((E, 128), np.float32)
        wv_p = np.zeros((E, HPC * 9), np.float32)
        wo_p = np.zeros((128, E), np.float32)
        for h in range(HPC):
            g = 4 * hh + h
            wq_p[:, 32 * h:32 * h + 8] = Wq[g]
            wk_p[:, 32 * h:32 * h + 8] = Wk[g]
            wv_p[:, 9 * h + 1:9 * h + 9] = Wv[g]
            wo_p[32 * h + 1:32 * h + 9, :] = Wo[8 * g:8 * g + 8, :]
        in_maps.append(
            {
                "qT": qTs[b], "kT": kTs[b], "vT": vTs[b],
                "wq": wq_p.astype(bf), "wk": wk_p.astype(bf),
                "wv": wv_p.astype(bf), "wo": wo_p.astype(bf),
                "msk": msk2,
            }
        )
    return in_maps


def _reference_numpy(query, key, value, padding_mask, decoder_mask,
                     Wq, Wk, Wv, Wo, bo):
    """Fallback (non-default masks): plain numpy replica of the reference."""
    q = np.einsum("bse,hed->bhsd", query, Wq)
    k = np.einsum("bse,hed->bhsd", key, Wk)
    v = np.einsum("bse,hed->bhsd", value, Wv)
    s = np.einsum("bhsd,bhtd->bhst", q, k)
    if decoder_mask:
        tril = np.tril(s)
        s = np.where(tril == 0.0, -np.inf, s)
    s = np.where(padding_mask[:, None, :, :], s, -np.inf)
    s = s / np.sqrt(np.float32(DK_H))
    m = np.max(s, axis=-1, keepdims=True)
    e = np.exp(s - m)
    a = e / np.sum(e, axis=-1, keepdims=True)
    o = np.einsum("bhst,bhtd->bhsd", a, v)
    o = o.transpose(0, 2, 1, 3).reshape(o.shape[0], o.shape[2], H * DV_H)
    return (o @ Wo + bo).astype(np.float32)


def kernel(query, key, value, padding_mask, decoder_mask, Wq, Wk, Wv, Wo, bo,
           **run_kwargs):
    query = np.asarray(query, np.float32)
    key = np.asarray(key, np.float32)
    value = np.asarray(value, np.float32)
    Wq = np.asarray(Wq, np.float32)
    Wk = np.asarray(Wk, np.float32)
    Wv = np.asarray(Wv, np.float32)
    Wo = np.asarray(Wo, np.float32)
    bo = np.asarray(bo, np.float32)
    pm = np.asarray(padding_mask)
    dm = int(np.asarray(decoder_mask))

    if not bool(pm.all()) or not dm:
        return _reference_numpy(
            query, key, value, pm.astype(bool), dm, Wq, Wk, Wv, Wo, bo
        )

    from concourse.bass_utils import run_bass_kernel_spmd

    if "nc" not in _cache:
        _cache["nc"] = _build()
    nc = _cache["nc"]

    in_maps = _prep_inputs(query, key, value, Wq, Wk, Wv, Wo)
    res = run_bass_kernel_spmd(nc, in_maps, list(range(NCORES)), **run_kwargs)

    outp = np.empty((B, S, E), np.float32)
    for b in range(B):
        fT = (res.results[2 * b]["out"].astype(np.float32)
              + res.results[2 * b + 1]["out"].astype(np.float32))
        outp[b] = fT.T + bo
    if run_kwargs:
        kernel.last_result = res
    return outp
